# revision 1
# baseline (speedup 1.0000x reference)
"""Trainium2 Bass kernel for nn_CausalSelfAttention_61795989455492.

Sharding (8 cores): core c -> batch b = c//2, head-group hg = c%2 (8 of 16
heads). Each core runs QKV projection (its head slice), rotary, sliding-window
attention with joint prefix softmax, and a partial output projection over its
512 channel columns. Host sums the two partials per batch (pair reduce).

Device layout ("transposed attention"):
  - q^T, k^T: (d on partition, t on free) straight out of projection matmuls
  - att^T blocks: (s on partition, t on free); softmax denominator comes from a
    ones-column appended to V (y_aug row 64), so no partition reductions needed
  - window mask folded into PSUM via tiny bf16 identity x mask matmuls
  - exp via ScalarE with scale=1/sqrt(D); no max subtraction (scores are O(5))
"""

import sys
from contextlib import ExitStack

import numpy as np

sys.path.insert(0, "/opt/trn_rl_repo")

import ml_dtypes  # noqa: E402
import concourse.bass as bass  # noqa: E402
import concourse.tile as tile_mod  # noqa: E402
from concourse import bacc  # noqa: E402
from concourse import mybir  # noqa: E402

B, T, C, H, D = 4, 512, 1024, 16, 64
S_PREV, PFX, WINDOW = 1536, 256, 256
ROPE_BASE = 10000.0
MASKVAL = -1.0e5
HPC = 8  # heads per core
NCORES = 8

f32 = mybir.dt.float32
f32r = mybir.dt.float32r
bf16 = mybir.dt.bfloat16

# window geometry per 512-col KV chunk, transposed layout:
# s-block tj -> t-run [T0[tj], T0[tj]+TN[tj])  (t-blocks ti in {tj,tj+1,tj+2})
_T0 = [0, 128, 256, 256]
_TN = [384, 384, 256, 256]
# psum column offset of each tj window inside the (128,1536) chunk tile
_POFF = [0, 512, 1024, 1280]
# exp-output column offset of each tj window inside the (128,1280) tile
_EOFF = [0, 384, 768, 1024]


def _emit(nc, tc, io):
    ctx = ExitStack()
    with ctx:
        const = ctx.enter_context(tc.tile_pool(name="const", bufs=1))
        qkrot = ctx.enter_context(tc.tile_pool(name="qkrot", bufs=1))
        vsb = ctx.enter_context(tc.tile_pool(name="vsb", bufs=1))
        ysb = ctx.enter_context(tc.tile_pool(name="ysb", bufs=1))
        tmp = ctx.enter_context(tc.tile_pool(name="tmp", bufs=3))

        sb_cos = const.tile([128, 512], f32)
        nc.sync.dma_start(out=sb_cos, in_=io["cos2"].ap())
        sb_sin = const.tile([128, 512], f32)
        nc.sync.dma_start(out=sb_sin, in_=io["sin2"].ap())
        sb_I = const.tile([128, 128], bf16)
        nc.sync.dma_start(out=sb_I, in_=io["ident"].ap())
        sb_diag = const.tile([128, 128], bf16)
        nc.sync.dma_start(out=sb_diag, in_=io["diag_tri"].ap())
        sb_bound = const.tile([128, 128], bf16)
        nc.sync.dma_start(out=sb_bound, in_=io["bound_tri"].ap())
        sb_full = const.tile([128, 128], bf16)
        nc.sync.dma_start(out=sb_full, in_=io["full_msk"].ap())
        ones1 = const.tile([1, 64], f32r)
        nc.sync.dma_start(out=ones1, in_=io["ones_row"].ap())
        ones4 = const.tile([128, 4, 1], f32r)
        nc.sync.dma_start(out=ones4, in_=io["ones4"].ap())

        q_rot = [qkrot.tile([128, 512], f32r, name=f"qrot{i}", tag=f"qrot{i}") for i in range(4)]
        k_rot = [qkrot.tile([128, 512], f32r, name=f"krot{i}", tag=f"krot{i}") for i in range(4)]
        v_sb = [vsb.tile([128, 512], f32, name=f"vsb{i}", tag=f"vsb{i}") for i in range(4)]
        y_t = [ysb.tile([128, 512], f32r, name=f"ysb{i}", tag=f"ysb{i}") for i in range(4)]

        # ---------------- phase 1: qkv projection + rotary ----------------
        with tc.tile_pool(name="wqkv", bufs=1) as wpool, \
             tc.tile_pool(name="xt", bufs=1) as xpool, \
             tc.tile_pool(name="projps", bufs=4, space="PSUM") as projps:
            sb_w = []
            for i in range(8):
                w = wpool.tile([128, 1536], f32r, name=f"w{i}", tag=f"w{i}")
                nc.sync.dma_start(out=w, in_=io["w_qkvT"].ap()[i * 128:(i + 1) * 128, :])
                sb_w.append(w)
            sb_x = []
            for i in range(8):
                xt = xpool.tile([128, 512], f32r, name=f"x{i}", tag=f"x{i}")
                nc.sync.dma_start(out=xt, in_=io["xT"].ap()[i * 128:(i + 1) * 128, :])
                sb_x.append(xt)

            # q^T and k^T: m-tiles 0..7 over qkv rows (q: 0..3, k: 4..7)
            for m in range(8):
                ps = projps.tile([128, 512], f32, name="projps", tag="projps")
                for c in range(8):
                    nc.tensor.matmul(
                        ps,
                        lhsT=sb_w[c][:, m * 128:(m + 1) * 128],
                        rhs=sb_x[c],
                        start=(c == 0),
                        stop=(c == 7),
                    )
                # rotary: rot = qk * cos2 + shuffle(qk) * sin2
                rot = q_rot[m] if m < 4 else k_rot[m - 4]
                qsb = tmp.tile([128, 512], f32, name="qsb", tag="qsb")
                nc.vector.tensor_copy(qsb, ps)
                sh = tmp.tile([128, 512], f32, name="sh", tag="sh")
                for dst, src in ((0, 32), (32, 0), (64, 96), (96, 64)):
                    nc.gpsimd.tensor_copy(out=sh[dst:dst + 32, :], in_=qsb[src:src + 32, :])
                nc.vector.tensor_mul(rot, qsb, sb_cos)
                nc.vector.tensor_mul(sh, sh, sb_sin)
                nc.vector.tensor_add(rot, rot, sh)

            # v natural: t-blocks 0..3 -> (t, head*64+d)
            for tb in range(4):
                ps = projps.tile([128, 512], f32, name="projps", tag="projps")
                for c in range(8):
                    nc.tensor.matmul(
                        ps,
                        lhsT=sb_x[c][:, tb * 128:(tb + 1) * 128],
                        rhs=sb_w[c][:, 1024:1536],
                        start=(c == 0),
                        stop=(c == 7),
                    )
                nc.vector.tensor_copy(v_sb[tb], ps)

        # ---------------- phase 2: attention per head ----------------
        with tc.tile_pool(name="kts", bufs=2) as kts_p, \
             tc.tile_pool(name="pref", bufs=2) as pref_p, \
             tc.tile_pool(name="cvn", bufs=2) as cvn_p, \
             tc.tile_pool(name="vaug", bufs=3) as vaug_p, \
             tc.tile_pool(name="expsb", bufs=2) as exp_p, \
             tc.tile_pool(name="exppref", bufs=2) as expp_p, \
             tc.tile_pool(name="rdn", bufs=2) as rdn_p, \
             tc.tile_pool(name="attps", bufs=2, space="PSUM") as attps_p, \
             tc.tile_pool(name="yaug", bufs=2, space="PSUM") as yaug_p:
            kts = None
            for h in range(HPC):
                hrow = (h % 2) * 64
                mt = h // 2
                if h % 2 == 0:
                    kts = kts_p.tile([128, 1536], f32r, name="kts", tag="kts")
                    nc.sync.dma_start(out=kts, in_=io["kT_cache"].ap()[h // 2])
                pref = pref_p.tile([128, 1024], f32, name="pref", tag="pref")
                nc.sync.dma_start(out=pref, in_=io["prefT"].ap()[h])
                cvn = cvn_p.tile([128, 2, 65], f32r, name="cvn", tag="cvn")
                nc.sync.dma_start(
                    out=cvn[:, :, 0:64],
                    in_=io["cache_v_n"].ap()[h].rearrange("(blk p) d -> p blk d", p=128),
                )
                nc.vector.tensor_copy(cvn[:, :, 64:65], ones4[:, 0:2, :])

                yps = yaug_p.tile([128, 512], f32, name="yaug", tag="yaug")

                # prefix: exp then AV (+denominator via ones column)
                expp = expp_p.tile([128, 1024], f32r, name="exppref", tag="exppref")
                nc.scalar.activation(out=expp, in_=pref, func=mybir.ActivationFunctionType.Exp)
                for pb in range(2):
                    nc.tensor.matmul(
                        yps[0:65, :],
                        lhsT=cvn[:, pb, :],
                        rhs=expp[:, pb * 512:(pb + 1) * 512],
                        start=(pb == 0),
                        stop=False,
                        skip_group_check=True,
                    )

                for ck in range(4):
                    aps = attps_p.tile([128, 1536], f32, name="attps", tag="attps")
                    vau = vaug_p.tile([128, 4, 65], f32r, name="vaug", tag="vaug")
                    if ck < 3:
                        nc.sync.dma_start(
                            out=vau[:, :, 0:64],
                            in_=io["v_cache"].ap()[h, ck * 512:(ck + 1) * 512, :]
                            .rearrange("(blk p) d -> p blk d", p=128),
                        )
                    else:
                        for tb in range(4):
                            nc.vector.tensor_copy(
                                vau[:, tb, 0:64], v_sb[tb][:, h * 64:(h + 1) * 64]
                            )
                    nc.vector.tensor_copy(vau[:, :, 64:65], ones4)

                    # QK + mask matmuls per s-block window
                    for tj in range(4):
                        t0, tn, off = _T0[tj], _TN[tj], _POFF[tj]
                        if ck < 3:
                            kblk = kts[hrow:hrow + 64, ck * 512 + tj * 128: ck * 512 + (tj + 1) * 128]
                        else:
                            kblk = k_rot[mt][hrow:hrow + 64, tj * 128:(tj + 1) * 128]
                        qs = q_rot[mt][hrow:hrow + 64, t0:t0 + tn]
                        nc.tensor.matmul(
                            aps[:, off:off + tn],
                            lhsT=kblk,
                            rhs=qs,
                            start=True,
                            stop=False,
                            skip_group_check=True,
                        )
                        dt0 = tj * 128 - t0  # diag block local offset
                        last_mask = (tj >= 2)
                        if tj == 3:  # fully-masked ti=2 block sits at local 0
                            nc.tensor.matmul(
                                aps[:, off:off + 128], lhsT=sb_I, rhs=sb_full,
                                start=False, stop=False, skip_group_check=True,
                            )
                        nc.tensor.matmul(
                            aps[:, off + dt0:off + dt0 + 128], lhsT=sb_I, rhs=sb_diag,
                            start=False, stop=last_mask, skip_group_check=True,
                        )
                        if tj < 2:  # boundary block ti=tj+2 at local 256
                            nc.tensor.matmul(
                                aps[:, off + 256:off + 384], lhsT=sb_I, rhs=sb_bound,
                                start=False, stop=True, skip_group_check=True,
                            )

                    # exp (scale=1/sqrt(D)); two instructions cover the 3 banks
                    ex = exp_p.tile([128, 1280], f32r, name="expsb", tag="expsb")
                    in01 = aps.rearrange("p (w c) -> p w c", c=512)[:, 0:2, 0:384]
                    out01 = ex[:, 0:768].rearrange("p (w c) -> p w c", c=384)
                    nc.scalar.activation(
                        out=out01, in_=in01, func=mybir.ActivationFunctionType.Exp,
                        scale=0.125,
                    )
                    nc.scalar.activation(
                        out=ex[:, 768:1280], in_=aps[:, 1024:1536],
                        func=mybir.ActivationFunctionType.Exp, scale=0.125,
                    )

                    # AV accumulate into y_aug
                    for tj in range(4):
                        t0, tn, eoff = _T0[tj], _TN[tj], _EOFF[tj]
                        nc.tensor.matmul(
                            yps[0:65, t0:t0 + tn],
                            lhsT=vau[:, tj, :],
                            rhs=ex[:, eoff:eoff + tn],
                            start=False,
                            stop=(ck == 3 and tj == 3),
                            skip_group_check=True,
                        )

                # normalize: y^T = y_aug[0:64] * (1/denom) broadcast over d
                rcp = rdn_p.tile([1, 512], f32r, name="rcp", tag="rcp")
                with nc.allow_low_precision(reason="fp32r reciprocal feeds broadcast matmul"):
                    nc.vector.reciprocal(rcp, yps[64:65, :])
                rbp = yaug_p.tile([64, 512], f32, name="rbp", tag="yaug")
                nc.tensor.matmul(rbp, lhsT=ones1, rhs=rcp,
                                 start=True, stop=True)
                rb = rdn_p.tile([64, 512], f32, name="rb", tag="rb")
                nc.vector.tensor_copy(rb, rbp)
                nc.vector.tensor_mul(y_t[mt][hrow:hrow + 64, :], yps[0:64, :], rb)

        # ---------------- phase 3: output projection (partial) ----------------
        with tc.tile_pool(name="wp", bufs=1) as wp_p, \
             tc.tile_pool(name="outsb", bufs=3) as out_p, \
             tc.tile_pool(name="cpps", bufs=3, space="PSUM") as cpps_p:
            wp = []
            for ct in range(4):
                w = wp_p.tile([128, 1024], f32r, name=f"wp{ct}", tag=f"wp{ct}")
                nc.sync.dma_start(out=w, in_=io["w_projT"].ap()[ct * 128:(ct + 1) * 128, :])
                wp.append(w)
            for tb in range(4):
                for ng in range(2):
                    cps = cpps_p.tile([128, 512], f32, name="cpps", tag="cpps")
                    for ct in range(4):
                        nc.tensor.matmul(
                            cps,
                            lhsT=y_t[ct][:, tb * 128:(tb + 1) * 128],
                            rhs=wp[ct][:, ng * 512:(ng + 1) * 512],
                            start=(ct == 0),
                            stop=(ct == 3),
                        )
                    ob = out_p.tile([128, 512], f32, name="outsb", tag="outsb")
                    nc.vector.tensor_copy(ob, cps)
                    nc.sync.dma_start(
                        out=io["out"].ap()[tb * 128:(tb + 1) * 128, ng * 512:(ng + 1) * 512],
                        in_=ob,
                    )


def build_nc():
    nc = bacc.Bacc("TRN2", target_bir_lowering=False, debug=False)
    io = {}
    io["xT"] = nc.declare_dram_parameter("xT", [1024, 512], f32r, isOutput=False)
    io["w_qkvT"] = nc.declare_dram_parameter("w_qkvT", [1024, 1536], f32r, isOutput=False)
    io["kT_cache"] = nc.declare_dram_parameter("kT_cache", [HPC // 2, 128, 1536], f32r, isOutput=False)
    io["v_cache"] = nc.declare_dram_parameter("v_cache", [HPC, 1536, 64], f32r, isOutput=False)
    io["prefT"] = nc.declare_dram_parameter("prefT", [HPC, 128, 1024], f32, isOutput=False)
    io["cache_v_n"] = nc.declare_dram_parameter("cache_v_n", [HPC, 256, 64], f32r, isOutput=False)
    io["w_projT"] = nc.declare_dram_parameter("w_projT", [512, 1024], f32r, isOutput=False)
    io["cos2"] = nc.declare_dram_parameter("cos2", [128, 512], f32, isOutput=False)
    io["sin2"] = nc.declare_dram_parameter("sin2", [128, 512], f32, isOutput=False)
    io["ident"] = nc.declare_dram_parameter("ident", [128, 128], bf16, isOutput=False)
    io["diag_tri"] = nc.declare_dram_parameter("diag_tri", [128, 128], bf16, isOutput=False)
    io["bound_tri"] = nc.declare_dram_parameter("bound_tri", [128, 128], bf16, isOutput=False)
    io["full_msk"] = nc.declare_dram_parameter("full_msk", [128, 128], bf16, isOutput=False)
    io["ones_row"] = nc.declare_dram_parameter("ones_row", [1, 64], f32r, isOutput=False)
    io["ones4"] = nc.declare_dram_parameter("ones4", [128, 4, 1], f32r, isOutput=False)
    io["out"] = nc.declare_dram_parameter("out", [512, 1024], f32, isOutput=True)

    with tile_mod.TileContext(nc) as tc:
        _emit(nc, tc, io)
    nc.finalize()
    return nc


def _rotary_tables(start_index):
    half = D // 2
    inv_freq = 1.0 / (ROPE_BASE ** (np.arange(half, dtype=np.float32) / half))
    pos = (float(start_index) + np.arange(T, dtype=np.float32))
    ang = inv_freq[:, None] * pos[None, :]  # (32, 512): [d, t]
    c = np.cos(ang, dtype=np.float32)
    s = np.sin(ang, dtype=np.float32)
    cos2 = np.tile(c, (4, 1))  # (128, 512)
    sin2 = np.tile(np.concatenate([-s, s], axis=0), (2, 1))  # (128, 512)
    return np.ascontiguousarray(cos2), np.ascontiguousarray(sin2)


def _mask_consts():
    ident = np.eye(128, dtype=ml_dtypes.bfloat16)
    i = np.arange(128)
    diag = np.where(i[:, None] > i[None, :], MASKVAL, 0.0).astype(ml_dtypes.bfloat16)
    bound = np.where(i[None, :] > i[:, None], MASKVAL, 0.0).astype(ml_dtypes.bfloat16)
    full = np.full((128, 128), MASKVAL, dtype=ml_dtypes.bfloat16)
    return ident, diag, bound, full


def make_in_maps(x, c_attn_w, c_proj_w, cached_k, cached_v, att_prefix, cache_v, start_index):
    cos2, sin2 = _rotary_tables(np.asarray(start_index).item())
    ident, diag, bound, full = _mask_consts()
    in_maps = []
    for core in range(NCORES):
        b, hg = core // 2, core % 2
        hs = slice(hg * HPC, (hg + 1) * HPC)
        r0, r1 = hg * 512, (hg + 1) * 512
        wq = c_attn_w[r0:r1]
        wk = c_attn_w[C + r0:C + r1]
        wv = c_attn_w[2 * C + r0:2 * C + r1]
        w_qkvT = np.ascontiguousarray(np.concatenate([wq, wk, wv], axis=0).T)
        p = att_prefix[b, hs].transpose(0, 2, 1)  # (8, 256, 512)
        prefT = np.ascontiguousarray(np.concatenate([p[:, :128], p[:, 128:]], axis=2))
        in_maps.append({
            "xT": np.ascontiguousarray(x[b].T),
            "w_qkvT": w_qkvT,
            "kT_cache": np.ascontiguousarray(
                cached_k[b, hs].transpose(0, 2, 1).reshape(HPC // 2, 128, 1536)),
            "v_cache": np.ascontiguousarray(cached_v[b, hs]),
            "prefT": prefT,
            "cache_v_n": np.ascontiguousarray(cache_v[b, hs]),
            "w_projT": np.ascontiguousarray(c_proj_w[:, r0:r1].T),
            "cos2": cos2,
            "sin2": sin2,
            "ident": ident,
            "ones_row": np.ones((1, 64), np.float32),
            "ones4": np.ones((128, 4, 1), np.float32),
            "diag_tri": diag,
            "bound_tri": bound,
            "full_msk": full,
        })
    return in_maps


_NC_CACHE = {}


def kernel(x, c_attn_w, c_proj_w, cached_k, cached_v, att_prefix, cache_v, start_index):
    x = np.asarray(x, dtype=np.float32)
    c_attn_w = np.asarray(c_attn_w, dtype=np.float32)
    c_proj_w = np.asarray(c_proj_w, dtype=np.float32)
    cached_k = np.asarray(cached_k, dtype=np.float32)
    cached_v = np.asarray(cached_v, dtype=np.float32)
    att_prefix = np.asarray(att_prefix, dtype=np.float32)
    cache_v = np.asarray(cache_v, dtype=np.float32)

    if "nc" not in _NC_CACHE:
        _NC_CACHE["nc"] = build_nc()
    nc = _NC_CACHE["nc"]

    in_maps = make_in_maps(x, c_attn_w, c_proj_w, cached_k, cached_v,
                           att_prefix, cache_v, start_index)
    from concourse.bass_utils import run_bass_kernel_spmd
    res = run_bass_kernel_spmd(nc, in_maps, list(range(NCORES)))
    outs = res.results
    y = np.empty((B, T, C), dtype=np.float32)
    for b in range(B):
        y[b] = outs[2 * b]["out"] + outs[2 * b + 1]["out"]
    return y



# revision 19
# speedup vs baseline: 1.9018x; 1.9018x over previous
"""Trainium2 Bass kernel for nn_CausalSelfAttention_61795989455492.

Sharding (8 cores): core c -> batch b = c//2, head-group hg = c%2 (8 of 16
heads). Each core runs QKV projection (its head slice), rotary, sliding-window
attention with joint prefix softmax, and a partial output projection over its
512 channel columns. Host sums the two partials per batch (pair reduce).

Device layout ("transposed attention", bf16 matmuls):
  - q^T, k^T: (d on partition, t on free) straight out of projection matmuls
  - att^T blocks: (s on partition, t on free); per-chunk windows are
    [0,384) [128,512) [256,512) [384,512) for s-blocks 0..3 (1152 cols)
  - window mask applied as a single DVE multiply on the bf16 exp output
  - softmax denominator via a ones-column appended to V (y_aug row 64);
    reciprocal_approx_fast + one broadcast matmul per head pair
  - exp via one ScalarE activation per chunk (scale=1/sqrt(D)); no max
    subtraction (scores are O(5))
"""

import sys
from contextlib import ExitStack

import numpy as np

sys.path.insert(0, "/opt/trn_rl_repo")

import ml_dtypes  # noqa: E402
import concourse.bass as bass  # noqa: E402
import concourse.tile as tile_mod  # noqa: E402
from concourse import bacc  # noqa: E402
from concourse import mybir  # noqa: E402

B, T, C, H, D = 4, 512, 1024, 16, 64
S_PREV, PFX, WINDOW = 1536, 256, 256
ROPE_BASE = 10000.0
HPC = 8  # heads per core
NCORES = 8

f32 = mybir.dt.float32
bf16 = mybir.dt.bfloat16
BF = ml_dtypes.bfloat16

# window geometry per 512-col KV chunk, transposed layout:
# s-block tj (rows u = tj*128+p) may see t in [u, u+256] within the chunk
_T0 = [0, 128, 256, 384]     # t-run start per tj
_TN = [384, 384, 256, 128]   # t-run length per tj
_POFF = [0, 512, 1024, 1280]  # psum col offset (bank-aligned; tj2/tj3 packed)
_EOFF = [0, 384, 768, 1024]   # exp-tile col offset

_MORDER = [0, 4, 1, 5, 2, 6, 3, 7]  # q0,k0,q1,k1,... for earliest head 0
MUL = mybir.AluOpType.mult
ADD = mybir.AluOpType.add
DEBUG_DUMP = False


def _emit(nc, tc, io):
    ctx = ExitStack()
    with ctx:
        const = ctx.enter_context(tc.tile_pool(name="const", bufs=1))
        qkrot = ctx.enter_context(tc.tile_pool(name="qkrot", bufs=1))
        vsb = ctx.enter_context(tc.tile_pool(name="vsb", bufs=1))
        ysb = ctx.enter_context(tc.tile_pool(name="ysb", bufs=1))
        ktsp = ctx.enter_context(tc.tile_pool(name="ktsp", bufs=1))
        vaup = ctx.enter_context(tc.tile_pool(name="vaup", bufs=1))
        prefp = ctx.enter_context(tc.tile_pool(name="prefp", bufs=1))
        cvnp = ctx.enter_context(tc.tile_pool(name="cvnp", bufs=1))
        wpp = ctx.enter_context(tc.tile_pool(name="wpp", bufs=1))

        # ---- attention-side bulk loads: scalar HWDGE queue, issued at t=0 ----
        kts = [ktsp.tile([128, 1536], bf16, name=f"kts{i}", tag=f"kts{i}")
               for i in range(4)]
        vau = [vaup.tile([128, 16, 65], bf16, name=f"vau{h}", tag=f"vau{h}")
               for h in range(HPC)]
        pref = [prefp.tile([128, 1024], bf16, name=f"pref{h}", tag=f"pref{h}")
                for h in range(HPC)]
        cvn = [cvnp.tile([128, 2, 65], bf16, name=f"cvn{h}", tag=f"cvn{h}")
               for h in range(HPC)]
        for h in range(HPC):
            if h % 2 == 0:
                nc.scalar.dma_start(out=kts[h // 2], in_=io["kT_cache"].ap()[h // 2])
            nc.scalar.dma_start(out=vau[h][:, 0:12, :], in_=io["v_pack"].ap()[h])
            nc.scalar.dma_start(out=pref[h], in_=io["prefT"].ap()[h])
            nc.scalar.dma_start(out=cvn[h], in_=io["cvn"].ap()[h])

        # ---- consts + phase-1/3 weights: sync HWDGE queue ----
        sb_cos = const.tile([128, 512], f32)
        nc.sync.dma_start(out=sb_cos, in_=io["cos2"].ap())
        sb_sin = const.tile([128, 512], f32)
        nc.sync.dma_start(out=sb_sin, in_=io["sin2"].ap())
        sb_mask = const.tile([128, 1152], bf16)
        nc.sync.dma_start(out=sb_mask, in_=io["maskw"].ap())
        sb_ones = const.tile([1, 64], bf16)
        nc.sync.dma_start(out=sb_ones, in_=io["ones64"].ap())

        q_rot = [qkrot.tile([128, 512], bf16, name=f"qrot{i}", tag=f"qrot{i}") for i in range(4)]
        k_rot = [qkrot.tile([128, 512], bf16, name=f"krot{i}", tag=f"krot{i}") for i in range(4)]
        v_sb = [vsb.tile([128, 512], bf16, name=f"vsb{i}", tag=f"vsb{i}") for i in range(4)]
        y_t = [ysb.tile([128, 512], bf16, name=f"ysb{i}", tag=f"ysb{i}") for i in range(4)]

        # ---------------- phase 1: qkv projection + rotary ----------------
        with tc.tile_pool(name="wqk", bufs=1) as wqkp, \
             tc.tile_pool(name="wv", bufs=1) as wvp, \
             tc.tile_pool(name="xt", bufs=1) as xpool, \
             tc.tile_pool(name="qsb", bufs=2) as qsbp, \
             tc.tile_pool(name="shb", bufs=2) as shp, \
             tc.tile_pool(name="rtmp", bufs=4) as rtmp, \
             tc.tile_pool(name="projps", bufs=3, space="PSUM") as projps:
            sb_x = [xpool.tile([128, 512], bf16, name=f"x{i}", tag=f"x{i}")
                    for i in range(8)]
            sb_wm = [wqkp.tile([128, 8, 128], bf16, name=f"wm{i}", tag=f"wm{i}")
                     for i in range(8)]
            sb_wv = [wvp.tile([128, 512], bf16, name=f"wv{i}", tag=f"wv{i}")
                     for i in range(8)]
            # interleave so the m=0 chain can start after x0+wm0
            nc.sync.dma_start(out=sb_x[0], in_=io["xT"].ap()[0:128, :])
            nc.sync.dma_start(out=sb_wm[0], in_=io["w_qk"].ap()[0])
            nc.sync.dma_start(out=sb_x[1], in_=io["xT"].ap()[128:256, :])
            nc.sync.dma_start(out=sb_wm[4], in_=io["w_qk"].ap()[4])
            for i in range(2, 8):
                nc.sync.dma_start(out=sb_x[i], in_=io["xT"].ap()[i * 128:(i + 1) * 128, :])
            for m in (1, 5, 2, 6, 3, 7):
                nc.sync.dma_start(out=sb_wm[m], in_=io["w_qk"].ap()[m])
            for i in range(8):
                nc.sync.dma_start(out=sb_wv[i], in_=io["w_v"].ap()[i])
            wp = []
            for ct in range(4):
                w = wpp.tile([128, 1024], bf16, name=f"wp{ct}", tag=f"wp{ct}")
                nc.sync.dma_start(out=w, in_=io["w_projT"].ap()[ct])
                wp.append(w)

            # q^T / k^T m-tiles with rotary
            for m in _MORDER:
                ps = projps.tile([128, 512], f32, name="projps", tag="projps")
                for c in range(8):
                    nc.tensor.matmul(
                        ps, lhsT=sb_wm[m][:, c, :], rhs=sb_x[c],
                        start=(c == 0), stop=(c == 7),
                    )
                qsb = qsbp.tile([128, 512], f32, name="qsb", tag="qsb")
                nc.scalar.activation(out=qsb, in_=ps,
                                     func=mybir.ActivationFunctionType.Copy)
                sh = shp.tile([128, 512], f32, name="shb", tag="shb")
                for blk in (0, 64):
                    nc.sync.dma_start(out=sh[blk + 32:blk + 64, :],
                                      in_=qsb[blk:blk + 32, :])
                    nc.sync.dma_start(out=sh[blk:blk + 32, :],
                                      in_=qsb[blk + 32:blk + 64, :])
                t1 = rtmp.tile([128, 512], f32, name="t1", tag="rt")
                nc.vector.scalar_tensor_tensor(
                    out=t1, in0=qsb, scalar=1.0, in1=sb_cos, op0=MUL, op1=MUL)
                t2 = rtmp.tile([128, 512], f32, name="t2", tag="rt")
                nc.vector.scalar_tensor_tensor(
                    out=t2, in0=sh, scalar=1.0, in1=sb_sin, op0=MUL, op1=MUL)
                dst = q_rot[m] if m < 4 else k_rot[m - 4]
                nc.vector.scalar_tensor_tensor(
                    out=dst, in0=t1, scalar=1.0, in1=t2, op0=MUL, op1=ADD)
                if DEBUG_DUMP and m in (0, 4):
                    nc.sync.dma_start(
                        out=io["dbg_qrot" if m == 0 else "dbg_krot"].ap(), in_=dst)

            # v natural: t-blocks 0..3 -> (t, head*64+d)
            for tb in range(4):
                ps = projps.tile([128, 512], f32, name="projps", tag="projps")
                for c in range(8):
                    nc.tensor.matmul(
                        ps, lhsT=sb_x[c][:, tb * 128:(tb + 1) * 128], rhs=sb_wv[c],
                        start=(c == 0), stop=(c == 7),
                    )
                nc.scalar.activation(out=v_sb[tb], in_=ps,
                                     func=mybir.ActivationFunctionType.Copy)

        # ---------------- phase 2: attention per head ----------------
        with tc.tile_pool(name="ep", bufs=2) as ep_p, \
             tc.tile_pool(name="exr", bufs=2) as exr_p, \
             tc.tile_pool(name="exm", bufs=2) as exm_p, \
             tc.tile_pool(name="rcf", bufs=4) as rcf_p, \
             tc.tile_pool(name="rcb", bufs=2) as rcb_p, \
             tc.tile_pool(name="rbx", bufs=2) as rbx_p, \
             tc.tile_pool(name="attps", bufs=2, space="PSUM") as attps_p, \
             tc.tile_pool(name="yaug", bufs=2, space="PSUM") as yaug_p:
            yps_prev = None
            rcpf = None
            for h in range(HPC):
                mt, hrow = h // 2, (h % 2) * 64
                # current-chunk v + denominator-ones into vau blocks 12..15
                for tb in range(4):
                    nc.vector.tensor_scalar(
                        out=vau[h][:, 12 + tb, 0:64],
                        in0=v_sb[tb][:, h * 64:(h + 1) * 64],
                        scalar1=1.0, scalar2=None, op0=MUL)
                nc.vector.memset(vau[h][:, 12:16, 64:65], 1.0)

                yps = yaug_p.tile([128, 512], f32, name="yaug", tag="yaug")

                # prefix: exp then AV (+denominator via ones column)
                ep = ep_p.tile([128, 1024], bf16, name="ep", tag="ep")
                nc.scalar.activation(out=ep, in_=pref[h],
                                     func=mybir.ActivationFunctionType.Exp)
                for pb in range(2):
                    nc.tensor.matmul(
                        yps[0:65, :], lhsT=cvn[h][:, pb, :],
                        rhs=ep[:, pb * 512:(pb + 1) * 512],
                        start=(pb == 0), stop=False, skip_group_check=True,
                    )

                for ck in range(4):
                    aps = attps_p.tile([128, 1536], f32, name="attps", tag="attps")
                    for tj in range(4):
                        if ck < 3:
                            kblk = kts[mt][hrow:hrow + 64,
                                           ck * 512 + tj * 128:ck * 512 + (tj + 1) * 128]
                        else:
                            kblk = k_rot[mt][hrow:hrow + 64, tj * 128:(tj + 1) * 128]
                        nc.tensor.matmul(
                            aps[:, _POFF[tj]:_POFF[tj] + _TN[tj]],
                            lhsT=kblk,
                            rhs=q_rot[mt][hrow:hrow + 64, _T0[tj]:_T0[tj] + _TN[tj]],
                            start=True, stop=True, skip_group_check=True,
                        )
                    # exp over the three 384-wide psum runs in one activation
                    exr = exr_p.tile([128, 1152], bf16, name="exr", tag="exr")
                    inap = aps.rearrange("p (w c) -> p w c", c=512)[:, 0:3, 0:384]
                    outap = exr.rearrange("p (w c) -> p w c", c=384)
                    nc.scalar.activation(
                        out=outap, in_=inap,
                        func=mybir.ActivationFunctionType.Exp, scale=0.125)
                    # window mask: one DVE multiply on the bf16 exp tile
                    exm = exm_p.tile([128, 1152], bf16, name="exm", tag="exm")
                    nc.vector.scalar_tensor_tensor(
                        out=exm, in0=exr, scalar=1.0, in1=sb_mask,
                        op0=MUL, op1=MUL)
                    if DEBUG_DUMP and h == 0 and ck == 0:
                        nc.sync.dma_start(out=io["dbg_exm"].ap(), in_=exm)
                        nc.sync.dma_start(out=io["dbg_exr"].ap(), in_=exr)
                    # AV accumulate into y_aug
                    for tj in range(4):
                        nc.tensor.matmul(
                            yps[0:65, _T0[tj]:_T0[tj] + _TN[tj]],
                            lhsT=vau[h][:, ck * 4 + tj, :],
                            rhs=exm[:, _EOFF[tj]:_EOFF[tj] + _TN[tj]],
                            start=False, stop=(ck == 3 and tj == 3),
                            skip_group_check=True,
                        )

                # denominator reciprocal, broadcast into yps[64:128] (unused
                # partitions of the same psum bank), normalize per head pair
                dsb = rcf_p.tile([1, 512], f32, name="dsb", tag="dsb")
                nc.scalar.activation(out=dsb, in_=yps[64:65, :],
                                     func=mybir.ActivationFunctionType.Copy)
                rcpf = rcf_p.tile([1, 512], f32, name="rcf", tag="rcf")
                nc.vector.reciprocal_approx_fast(out=rcpf, in_=dsb)
                rcpb = rcb_p.tile([1, 512], bf16, name="rcb", tag="rcb")
                nc.vector.tensor_scalar(
                    out=rcpb, in0=rcpf, scalar1=1.0, scalar2=None, op0=MUL)
                nc.tensor.matmul(yps[64:128, :], lhsT=sb_ones, rhs=rcpb,
                                 start=True, stop=True, skip_group_check=True)
                rb = rbx_p.tile([64, 512], bf16, name="rb", tag="rb")
                nc.scalar.activation(out=rb, in_=yps[64:128, :],
                                     func=mybir.ActivationFunctionType.Copy)
                if DEBUG_DUMP:
                    nc.sync.dma_start(out=io["dbg_rcp"].ap()[h], in_=rcpf)
                    nc.sync.dma_start(out=io["dbg_den"].ap()[h], in_=dsb)
                    nc.sync.dma_start(out=io["dbg_rb"].ap()[h], in_=rb[0:1, :])
                if h % 2 == 0:
                    yps_prev, rb_prev = yps, rb
                else:
                    nc.vector.scalar_tensor_tensor(
                        out=y_t[mt][0:64, :], in0=yps_prev[0:64, :], scalar=1.0,
                        in1=rb_prev, op0=MUL, op1=MUL)
                    nc.vector.scalar_tensor_tensor(
                        out=y_t[mt][64:128, :], in0=yps[0:64, :], scalar=1.0,
                        in1=rb, op0=MUL, op1=MUL)
                    if DEBUG_DUMP and h == 1:
                        nc.sync.dma_start(out=io["dbg_yt"].ap(), in_=y_t[0])

        # ---------------- phase 3: output projection (partial) ----------------
        with tc.tile_pool(name="outsb", bufs=3) as out_p, \
             tc.tile_pool(name="cpps", bufs=3, space="PSUM") as cpps_p:
            for tb in range(4):
                for ng in range(2):
                    cps = cpps_p.tile([128, 512], f32, name="cpps", tag="cpps")
                    for ct in range(4):
                        nc.tensor.matmul(
                            cps,
                            lhsT=y_t[ct][:, tb * 128:(tb + 1) * 128],
                            rhs=wp[ct][:, ng * 512:(ng + 1) * 512],
                            start=(ct == 0), stop=(ct == 3),
                        )
                    ob = out_p.tile([128, 512], bf16, name="outsb", tag="outsb")
                    nc.scalar.activation(out=ob, in_=cps,
                                         func=mybir.ActivationFunctionType.Copy)
                    nc.sync.dma_start(
                        out=io["out"].ap()[tb * 128:(tb + 1) * 128, ng * 512:(ng + 1) * 512],
                        in_=ob,
                    )


def build_nc():
    nc = bacc.Bacc("TRN2", target_bir_lowering=False, debug=False)
    io = {}
    io["xT"] = nc.declare_dram_parameter("xT", [1024, 512], bf16, isOutput=False)
    io["w_qk"] = nc.declare_dram_parameter("w_qk", [8, 128, 8, 128], bf16, isOutput=False)
    io["w_v"] = nc.declare_dram_parameter("w_v", [8, 128, 512], bf16, isOutput=False)
    io["kT_cache"] = nc.declare_dram_parameter("kT_cache", [4, 128, 1536], bf16, isOutput=False)
    io["v_pack"] = nc.declare_dram_parameter("v_pack", [HPC, 128, 780], bf16, isOutput=False)
    io["prefT"] = nc.declare_dram_parameter("prefT", [HPC, 128, 1024], bf16, isOutput=False)
    io["cvn"] = nc.declare_dram_parameter("cvn", [HPC, 128, 2, 65], bf16, isOutput=False)
    io["w_projT"] = nc.declare_dram_parameter("w_projT", [4, 128, 1024], bf16, isOutput=False)
    io["cos2"] = nc.declare_dram_parameter("cos2", [128, 512], f32, isOutput=False)
    io["sin2"] = nc.declare_dram_parameter("sin2", [128, 512], f32, isOutput=False)
    io["maskw"] = nc.declare_dram_parameter("maskw", [128, 1152], bf16, isOutput=False)
    io["ones64"] = nc.declare_dram_parameter("ones64", [1, 64], bf16, isOutput=False)
    io["out"] = nc.declare_dram_parameter("out", [512, 1024], bf16, isOutput=True)
    if DEBUG_DUMP:
        io["dbg_rcp"] = nc.declare_dram_parameter("dbg_rcp", [8, 512], f32, isOutput=True)
        io["dbg_den"] = nc.declare_dram_parameter("dbg_den", [8, 512], f32, isOutput=True)
        io["dbg_rb"] = nc.declare_dram_parameter("dbg_rb", [8, 512], bf16, isOutput=True)
        io["dbg_exm"] = nc.declare_dram_parameter("dbg_exm", [128, 1152], bf16, isOutput=True)
        io["dbg_exr"] = nc.declare_dram_parameter("dbg_exr", [128, 1152], bf16, isOutput=True)
        io["dbg_qrot"] = nc.declare_dram_parameter("dbg_qrot", [128, 512], bf16, isOutput=True)
        io["dbg_krot"] = nc.declare_dram_parameter("dbg_krot", [128, 512], bf16, isOutput=True)
        io["dbg_yt"] = nc.declare_dram_parameter("dbg_yt", [128, 512], bf16, isOutput=True)

    with tile_mod.TileContext(nc) as tc:
        _emit(nc, tc, io)
    nc.finalize()
    return nc


def _rotary_tables(start_index):
    half = D // 2
    inv_freq = 1.0 / (ROPE_BASE ** (np.arange(half, dtype=np.float32) / half))
    pos = (float(start_index) + np.arange(T, dtype=np.float32))
    ang = inv_freq[:, None] * pos[None, :]  # (32, 512): [d, t]
    c = np.cos(ang, dtype=np.float32)
    s = np.sin(ang, dtype=np.float32)
    cos2 = np.tile(c, (4, 1))  # (128, 512)
    sin2 = np.tile(np.concatenate([-s, s], axis=0), (2, 1))  # (128, 512)
    return np.ascontiguousarray(cos2), np.ascontiguousarray(sin2)


def _mask_const():
    p = np.arange(128)[:, None]
    c384 = np.arange(384)[None, :]
    tri384 = ((c384 >= p) & (c384 <= p + 256)).astype(np.float32)
    ltri256 = (np.arange(256)[None, :] >= p).astype(np.float32)
    ltri128 = (np.arange(128)[None, :] >= p).astype(np.float32)
    return np.ascontiguousarray(
        np.concatenate([tri384, tri384, ltri256, ltri128], axis=1)).astype(BF)


def make_in_maps(x, c_attn_w, c_proj_w, cached_k, cached_v, att_prefix, cache_v, start_index):
    cos2, sin2 = _rotary_tables(np.asarray(start_index).item())
    maskw = _mask_const()
    ones64 = np.ones((1, 64), dtype=BF)
    in_maps = []
    for core in range(NCORES):
        b, hg = core // 2, core % 2
        hs = slice(hg * HPC, (hg + 1) * HPC)
        r0, r1 = hg * 512, (hg + 1) * 512
        wq = c_attn_w[r0:r1]
        wk = c_attn_w[C + r0:C + r1]
        wv = c_attn_w[2 * C + r0:2 * C + r1]
        w_qk = np.empty((8, 128, 8, 128), dtype=BF)
        for m in range(4):
            w_qk[m] = wq[m * 128:(m + 1) * 128].T.reshape(8, 128, 128).transpose(1, 0, 2)
            w_qk[4 + m] = wk[m * 128:(m + 1) * 128].T.reshape(8, 128, 128).transpose(1, 0, 2)
        w_v = np.ascontiguousarray(wv.T.reshape(8, 128, 512)).astype(BF)

        kk = cached_k[b, hs]  # (8, 1536, 64)
        kT = np.empty((4, 128, 1536), dtype=BF)
        for pr in range(4):
            kT[pr, 0:64] = kk[2 * pr].T
            kT[pr, 64:128] = kk[2 * pr + 1].T

        vp = cached_v[b, hs].reshape(8, 12, 128, 64).transpose(0, 2, 1, 3)
        v_pack = np.concatenate(
            [vp, np.ones((8, 128, 12, 1), np.float32)], axis=3
        ).reshape(8, 128, 780).astype(BF)

        p_ = att_prefix[b, hs].transpose(0, 2, 1)  # (8, 256, 512)
        prefT = np.ascontiguousarray(
            np.concatenate([p_[:, :128], p_[:, 128:]], axis=2)).astype(BF)

        cv = cache_v[b, hs].reshape(8, 2, 128, 64).transpose(0, 2, 1, 3)
        cvn = np.concatenate(
            [cv, np.ones((8, 128, 2, 1), np.float32)], axis=3).astype(BF)

        w_projT = np.ascontiguousarray(
            c_proj_w[:, r0:r1].T.reshape(4, 128, 1024)).astype(BF)

        in_maps.append({
            "xT": np.ascontiguousarray(x[b].T).astype(BF),
            "w_qk": w_qk,
            "w_v": w_v,
            "kT_cache": kT,
            "v_pack": v_pack,
            "prefT": prefT,
            "cvn": cvn,
            "w_projT": w_projT,
            "cos2": cos2,
            "sin2": sin2,
            "maskw": maskw,
            "ones64": ones64,
        })
    return in_maps


_NC_CACHE = {}


def kernel(x, c_attn_w, c_proj_w, cached_k, cached_v, att_prefix, cache_v, start_index):
    x = np.asarray(x, dtype=np.float32)
    c_attn_w = np.asarray(c_attn_w, dtype=np.float32)
    c_proj_w = np.asarray(c_proj_w, dtype=np.float32)
    cached_k = np.asarray(cached_k, dtype=np.float32)
    cached_v = np.asarray(cached_v, dtype=np.float32)
    att_prefix = np.asarray(att_prefix, dtype=np.float32)
    cache_v = np.asarray(cache_v, dtype=np.float32)

    if "nc" not in _NC_CACHE:
        _NC_CACHE["nc"] = build_nc()
    nc = _NC_CACHE["nc"]

    in_maps = make_in_maps(x, c_attn_w, c_proj_w, cached_k, cached_v,
                           att_prefix, cache_v, start_index)
    from concourse.bass_utils import run_bass_kernel_spmd
    res = run_bass_kernel_spmd(nc, in_maps, list(range(NCORES)))
    outs = res.results
    y = np.empty((B, T, C), dtype=np.float32)
    for b in range(B):
        y[b] = (outs[2 * b]["out"].astype(np.float32)
                + outs[2 * b + 1]["out"].astype(np.float32))
    return y


# revision 20
# speedup vs baseline: 2.1505x; 1.1307x over previous
"""Trainium2 Bass kernel for nn_CausalSelfAttention_61795989455492.

Sharding (8 cores): core c -> batch b = c//2, head-group hg = c%2 (8 of 16
heads). Each core runs QKV projection (its head slice), rotary, sliding-window
attention with joint prefix softmax, and a partial output projection over its
512 channel columns. Host sums the two partials per batch (pair reduce).

Device layout ("transposed attention", bf16 matmuls):
  - q^T, k^T: (d on partition, t on free) straight out of projection matmuls;
    rotary runs on groups of 4 m-tiles (psum copy -> sbuf, DMA partition
    shuffle, 3 DVE multiplies against grouped cos/sin tables)
  - att^T blocks: (s on partition, t on free); per-chunk windows are
    [0,384) [128,512) [256,512) [384,512) for s-blocks 0..3 (1152 cols)
  - window mask applied as a single DVE multiply on the bf16 exp output
  - softmax denominator via a ones-column appended to V (y_aug row 64);
    reciprocal_approx_fast (input staged to SBUF - the custom DVE op reads
    PSUM wrong on HW) + a broadcast matmul into yps[64:128]
  - exp via one ScalarE activation per chunk (scale=1/sqrt(D)); no max
    subtraction (scores are O(5))
  - all bulk inputs land via one merged DMA per tensor (DMA push costs
    ~600ns of queue-engine time; fewer, bigger descriptors)
"""

import sys
from contextlib import ExitStack

import numpy as np

sys.path.insert(0, "/opt/trn_rl_repo")

import ml_dtypes  # noqa: E402
import concourse.bass as bass  # noqa: E402
import concourse.tile as tile_mod  # noqa: E402
from concourse import bacc  # noqa: E402
from concourse import mybir  # noqa: E402

B, T, C, H, D = 4, 512, 1024, 16, 64
S_PREV, PFX, WINDOW = 1536, 256, 256
ROPE_BASE = 10000.0
HPC = 8  # heads per core
NCORES = 8

f32 = mybir.dt.float32
bf16 = mybir.dt.bfloat16
BF = ml_dtypes.bfloat16

# window geometry per 512-col KV chunk, transposed layout:
# s-block tj (rows u = tj*128+p) may see t in [u, u+256] within the chunk
_T0 = [0, 128, 256, 384]     # t-run start per tj
_TN = [384, 384, 256, 128]   # t-run length per tj
_POFF = [0, 512, 1024, 1280]  # psum col offset (bank-aligned; tj2/tj3 packed)
_EOFF = [0, 384, 768, 1024]   # exp-tile col offset

_MORDER = [0, 4, 1, 5, 2, 6, 3, 7]  # q0,k0,q1,k1,... for earliest head 0
MUL = mybir.AluOpType.mult
ADD = mybir.AluOpType.add
DEBUG_DUMP = False


def _emit(nc, tc, io):
    ctx = ExitStack()
    with ctx:
        const = ctx.enter_context(tc.tile_pool(name="const", bufs=1))
        qkrot = ctx.enter_context(tc.tile_pool(name="qkrot", bufs=1))
        vsb = ctx.enter_context(tc.tile_pool(name="vsb", bufs=1))
        ysb = ctx.enter_context(tc.tile_pool(name="ysb", bufs=1))
        ktsp = ctx.enter_context(tc.tile_pool(name="ktsp", bufs=1))
        vaup = ctx.enter_context(tc.tile_pool(name="vaup", bufs=1))
        prefp = ctx.enter_context(tc.tile_pool(name="prefp", bufs=1))
        cvnp = ctx.enter_context(tc.tile_pool(name="cvnp", bufs=1))
        wpp = ctx.enter_context(tc.tile_pool(name="wpp", bufs=1))

        # ---- attention-side bulk loads: scalar HWDGE queue, one DMA each ----
        kts = ktsp.tile([128, 4, 1536], bf16, name="kts", tag="kts")
        nc.scalar.dma_start(out=kts, in_=io["kT_cache"].ap())
        cvn = cvnp.tile([128, 8, 2, 65], bf16, name="cvn", tag="cvn")
        nc.scalar.dma_start(out=cvn, in_=io["cvn"].ap())
        pref = prefp.tile([128, 8, 1024], bf16, name="pref", tag="pref")
        nc.scalar.dma_start(out=pref, in_=io["prefT"].ap())
        vau = vaup.tile([128, 8, 16, 65], bf16, name="vau", tag="vau")
        nc.scalar.dma_start(out=vau, in_=io["v_pack"].ap())

        q_g = [qkrot.tile([128, 2048], bf16, name=f"qkg{g}", tag=f"qkg{g}")
               for g in range(2)]

        def q_ap(mt):
            c0 = (2 * (mt % 2)) * 512
            return q_g[mt // 2][:, c0:c0 + 512]

        def k_ap(mt):
            c0 = (2 * (mt % 2) + 1) * 512
            return q_g[mt // 2][:, c0:c0 + 512]

        vsall = vsb.tile([128, 4, 512], bf16, name="vsall", tag="vsall")
        y_t = [ysb.tile([128, 512], bf16, name=f"ysb{i}", tag=f"ysb{i}") for i in range(4)]

        # ---------------- phase 1: qkv projection + rotary ----------------
        with tc.tile_pool(name="wqk", bufs=1) as wqkp, \
             tc.tile_pool(name="wv", bufs=1) as wvp, \
             tc.tile_pool(name="xt", bufs=1) as xpool, \
             tc.tile_pool(name="qsb", bufs=2) as qsbp, \
             tc.tile_pool(name="shb", bufs=2) as shp, \
             tc.tile_pool(name="rtmp", bufs=2) as rtmp, \
             tc.tile_pool(name="projps", bufs=3, space="PSUM") as projps:
            sb_x = xpool.tile([128, 8, 512], bf16, name="xall", tag="xall")
            nc.sync.dma_start(out=sb_x, in_=io["xT"].ap())
            wqa = wqkp.tile([128, 2, 8, 128], bf16, name="wqa", tag="wqa")
            nc.sync.dma_start(out=wqa, in_=io["w_qka"].ap())
            wqb = wqkp.tile([128, 6, 8, 128], bf16, name="wqb", tag="wqb")
            nc.sync.dma_start(out=wqb, in_=io["w_qkb"].ap())
            sb_wv = wvp.tile([128, 8, 512], bf16, name="wvall", tag="wvall")
            nc.sync.dma_start(out=sb_wv, in_=io["w_v"].ap())
            sb_cos = const.tile([128, 2048], bf16)
            nc.sync.dma_start(out=sb_cos, in_=io["cos_g"].ap())
            sb_sin = const.tile([128, 2048], bf16)
            nc.sync.dma_start(out=sb_sin, in_=io["sin_g"].ap())
            sb_mask = const.tile([128, 1152], bf16)
            nc.sync.dma_start(out=sb_mask, in_=io["maskw"].ap())
            sb_ones = const.tile([1, 64], bf16)
            nc.sync.dma_start(out=sb_ones, in_=io["ones64"].ap())
            wp = wpp.tile([128, 4, 1024], bf16, name="wpall", tag="wpall")
            nc.sync.dma_start(out=wp, in_=io["w_projT"].ap())

            def wm_ap(g, j):  # m-tile _MORDER[4g+j], contraction tile c slice
                return (wqa[:, 2 * g + j] if 4 * g + j < 2
                        else wqb[:, 4 * g + j - 2])

            # q^T / k^T in 2 groups of 4 m-tiles, rotary per group
            for g in range(2):
                qsb2 = qsbp.tile([128, 2048], bf16, name="qsb2", tag="qsb2")
                for j in range(4):
                    ps = projps.tile([128, 512], f32, name="projps", tag="projps")
                    wm = wm_ap(g, j)
                    for c in range(8):
                        nc.tensor.matmul(
                            ps, lhsT=wm[:, c, :], rhs=sb_x[:, c, :],
                            start=(c == 0), stop=(c == 7),
                        )
                    nc.scalar.activation(
                        out=qsb2[:, j * 512:(j + 1) * 512], in_=ps,
                        func=mybir.ActivationFunctionType.Copy)
                sh2 = shp.tile([128, 2048], bf16, name="sh2", tag="sh2")
                for blk in (0, 64):
                    nc.sync.dma_start(out=sh2[blk + 32:blk + 64, :],
                                      in_=qsb2[blk:blk + 32, :])
                    nc.sync.dma_start(out=sh2[blk:blk + 32, :],
                                      in_=qsb2[blk + 32:blk + 64, :])
                t1 = rtmp.tile([128, 2048], bf16, name="t1", tag="rt")
                nc.vector.scalar_tensor_tensor(
                    out=t1, in0=qsb2, scalar=1.0, in1=sb_cos, op0=MUL, op1=MUL)
                t2 = rtmp.tile([128, 2048], bf16, name="t2", tag="rt")
                nc.vector.scalar_tensor_tensor(
                    out=t2, in0=sh2, scalar=1.0, in1=sb_sin, op0=MUL, op1=MUL)
                nc.vector.scalar_tensor_tensor(
                    out=q_g[g], in0=t1, scalar=1.0, in1=t2, op0=MUL, op1=ADD)

            # v natural: t-blocks 0..3 -> (t, head*64+d)
            for tb in range(4):
                ps = projps.tile([128, 512], f32, name="projps", tag="projps")
                for c in range(8):
                    nc.tensor.matmul(
                        ps, lhsT=sb_x[:, c, tb * 128:(tb + 1) * 128],
                        rhs=sb_wv[:, c, :],
                        start=(c == 0), stop=(c == 7),
                    )
                nc.scalar.activation(out=vsall[:, tb, :], in_=ps,
                                     func=mybir.ActivationFunctionType.Copy)

        # ---------------- phase 2: attention per head ----------------
        with tc.tile_pool(name="ep", bufs=2) as ep_p, \
             tc.tile_pool(name="exr", bufs=2) as exr_p, \
             tc.tile_pool(name="exm", bufs=2) as exm_p, \
             tc.tile_pool(name="rcf", bufs=4) as rcf_p, \
             tc.tile_pool(name="rcb", bufs=2) as rcb_p, \
             tc.tile_pool(name="rbx", bufs=2) as rbx_p, \
             tc.tile_pool(name="attps", bufs=2, space="PSUM") as attps_p, \
             tc.tile_pool(name="yaug", bufs=2, space="PSUM") as yaug_p:
            yps_prev = rb_prev = ep = None
            for h in range(HPC):
                mt, hrow = h // 2, (h % 2) * 64
                # current-chunk v into vau blocks 12..15 (ones pre-set by DMA)
                nc.vector.tensor_scalar(
                    out=vau[:, h, 12:16, 0:64],
                    in0=vsall[:, :, h * 64:(h + 1) * 64],
                    scalar1=1.0, scalar2=None, op0=MUL)

                yps = yaug_p.tile([128, 512], f32, name="yaug", tag="yaug")

                # prefix: exp (one activation per head pair) then AV
                if h % 2 == 0:
                    ep = ep_p.tile([128, 2, 1024], bf16, name="ep", tag="ep")
                    nc.scalar.activation(out=ep, in_=pref[:, h:h + 2, :],
                                         func=mybir.ActivationFunctionType.Exp)
                for pb in range(2):
                    nc.tensor.matmul(
                        yps[0:65, :], lhsT=cvn[:, h, pb, :],
                        rhs=ep[:, h % 2, pb * 512:(pb + 1) * 512],
                        start=(pb == 0), stop=False, skip_group_check=True,
                    )

                for ck in range(4):
                    aps = attps_p.tile([128, 1536], f32, name="attps", tag="attps")
                    for tj in range(4):
                        if ck < 3:
                            kblk = kts[hrow:hrow + 64, mt,
                                       ck * 512 + tj * 128:ck * 512 + (tj + 1) * 128]
                        else:
                            kblk = k_ap(mt)[hrow:hrow + 64, tj * 128:(tj + 1) * 128]
                        nc.tensor.matmul(
                            aps[:, _POFF[tj]:_POFF[tj] + _TN[tj]],
                            lhsT=kblk,
                            rhs=q_ap(mt)[hrow:hrow + 64, _T0[tj]:_T0[tj] + _TN[tj]],
                            start=True, stop=True, skip_group_check=True,
                        )
                    # exp over the three 384-wide psum runs in one activation
                    exr = exr_p.tile([128, 1152], bf16, name="exr", tag="exr")
                    inap = aps.rearrange("p (w c) -> p w c", c=512)[:, 0:3, 0:384]
                    outap = exr.rearrange("p (w c) -> p w c", c=384)
                    nc.scalar.activation(
                        out=outap, in_=inap,
                        func=mybir.ActivationFunctionType.Exp, scale=0.125)
                    # window mask: one DVE multiply on the bf16 exp tile
                    exm = exm_p.tile([128, 1152], bf16, name="exm", tag="exm")
                    nc.vector.scalar_tensor_tensor(
                        out=exm, in0=exr, scalar=1.0, in1=sb_mask,
                        op0=MUL, op1=MUL)
                    # AV accumulate into y_aug
                    for tj in range(4):
                        nc.tensor.matmul(
                            yps[0:65, _T0[tj]:_T0[tj] + _TN[tj]],
                            lhsT=vau[:, h, ck * 4 + tj, :],
                            rhs=exm[:, _EOFF[tj]:_EOFF[tj] + _TN[tj]],
                            start=False, stop=(ck == 3 and tj == 3),
                            skip_group_check=True,
                        )

                # denominator: stage to SBUF (custom-DVE PSUM read is broken
                # on HW), reciprocal, broadcast into yps[64:128], stage back
                dsb = rcf_p.tile([1, 512], f32, name="dsb", tag="dsb")
                nc.scalar.activation(out=dsb, in_=yps[64:65, :],
                                     func=mybir.ActivationFunctionType.Copy)
                rcpf = rcf_p.tile([1, 512], f32, name="rcf", tag="rcf")
                nc.vector.reciprocal_approx_fast(out=rcpf, in_=dsb)
                rcpb = rcb_p.tile([1, 512], bf16, name="rcb", tag="rcb")
                nc.vector.tensor_scalar(
                    out=rcpb, in0=rcpf, scalar1=1.0, scalar2=None, op0=MUL)
                nc.tensor.matmul(yps[64:128, :], lhsT=sb_ones, rhs=rcpb,
                                 start=True, stop=True, skip_group_check=True)
                rb = rbx_p.tile([64, 512], bf16, name="rb", tag="rb")
                nc.scalar.activation(out=rb, in_=yps[64:128, :],
                                     func=mybir.ActivationFunctionType.Copy)
                if DEBUG_DUMP:
                    nc.sync.dma_start(out=io["dbg_rcp"].ap()[h], in_=rcpf)
                    nc.sync.dma_start(out=io["dbg_den"].ap()[h], in_=dsb)
                if h % 2 == 0:
                    yps_prev, rb_prev = yps, rb
                else:
                    nc.vector.scalar_tensor_tensor(
                        out=y_t[mt][0:64, :], in0=yps_prev[0:64, :], scalar=1.0,
                        in1=rb_prev, op0=MUL, op1=MUL)
                    nc.vector.scalar_tensor_tensor(
                        out=y_t[mt][64:128, :], in0=yps[0:64, :], scalar=1.0,
                        in1=rb, op0=MUL, op1=MUL)

        # ---------------- phase 3: output projection (partial) ----------------
        with tc.tile_pool(name="outsb", bufs=2) as out_p, \
             tc.tile_pool(name="cpps", bufs=3, space="PSUM") as cpps_p:
            for tb in range(4):
                obt = out_p.tile([128, 1024], bf16, name="outsb", tag="outsb")
                for ng in range(2):
                    cps = cpps_p.tile([128, 512], f32, name="cpps", tag="cpps")
                    for ct in range(4):
                        nc.tensor.matmul(
                            cps,
                            lhsT=y_t[ct][:, tb * 128:(tb + 1) * 128],
                            rhs=wp[:, ct, ng * 512:(ng + 1) * 512],
                            start=(ct == 0), stop=(ct == 3),
                        )
                    nc.scalar.activation(out=obt[:, ng * 512:(ng + 1) * 512],
                                         in_=cps,
                                         func=mybir.ActivationFunctionType.Copy)
                nc.sync.dma_start(
                    out=io["out"].ap()[tb * 128:(tb + 1) * 128, :], in_=obt)


def build_nc():
    nc = bacc.Bacc("TRN2", target_bir_lowering=False, debug=False)
    io = {}
    io["xT"] = nc.declare_dram_parameter("xT", [128, 8, 512], bf16, isOutput=False)
    io["w_qka"] = nc.declare_dram_parameter("w_qka", [128, 2, 8, 128], bf16, isOutput=False)
    io["w_qkb"] = nc.declare_dram_parameter("w_qkb", [128, 6, 8, 128], bf16, isOutput=False)
    io["w_v"] = nc.declare_dram_parameter("w_v", [128, 8, 512], bf16, isOutput=False)
    io["kT_cache"] = nc.declare_dram_parameter("kT_cache", [128, 4, 1536], bf16, isOutput=False)
    io["v_pack"] = nc.declare_dram_parameter("v_pack", [128, 8, 1040], bf16, isOutput=False)
    io["prefT"] = nc.declare_dram_parameter("prefT", [128, 8, 1024], bf16, isOutput=False)
    io["cvn"] = nc.declare_dram_parameter("cvn", [128, 8, 2, 65], bf16, isOutput=False)
    io["w_projT"] = nc.declare_dram_parameter("w_projT", [128, 4, 1024], bf16, isOutput=False)
    io["cos_g"] = nc.declare_dram_parameter("cos_g", [128, 2048], bf16, isOutput=False)
    io["sin_g"] = nc.declare_dram_parameter("sin_g", [128, 2048], bf16, isOutput=False)
    io["maskw"] = nc.declare_dram_parameter("maskw", [128, 1152], bf16, isOutput=False)
    io["ones64"] = nc.declare_dram_parameter("ones64", [1, 64], bf16, isOutput=False)
    io["out"] = nc.declare_dram_parameter("out", [512, 1024], bf16, isOutput=True)
    if DEBUG_DUMP:
        io["dbg_rcp"] = nc.declare_dram_parameter("dbg_rcp", [8, 512], f32, isOutput=True)
        io["dbg_den"] = nc.declare_dram_parameter("dbg_den", [8, 512], f32, isOutput=True)

    with tile_mod.TileContext(nc) as tc:
        _emit(nc, tc, io)
    nc.finalize()
    return nc


def _rotary_tables(start_index):
    half = D // 2
    inv_freq = 1.0 / (ROPE_BASE ** (np.arange(half, dtype=np.float32) / half))
    pos = (float(start_index) + np.arange(T, dtype=np.float32))
    ang = inv_freq[:, None] * pos[None, :]  # (32, 512): [d, t]
    c = np.cos(ang, dtype=np.float32)
    s = np.sin(ang, dtype=np.float32)
    cos2 = np.tile(c, (4, 1))  # (128, 512)
    sin2 = np.tile(np.concatenate([-s, s], axis=0), (2, 1))  # (128, 512)
    cos_g = np.ascontiguousarray(np.tile(cos2, (1, 4))).astype(BF)
    sin_g = np.ascontiguousarray(np.tile(sin2, (1, 4))).astype(BF)
    return cos_g, sin_g


def _mask_const():
    p = np.arange(128)[:, None]
    c384 = np.arange(384)[None, :]
    tri384 = ((c384 >= p) & (c384 <= p + 256)).astype(np.float32)
    ltri256 = (np.arange(256)[None, :] >= p).astype(np.float32)
    ltri128 = (np.arange(128)[None, :] >= p).astype(np.float32)
    return np.ascontiguousarray(
        np.concatenate([tri384, tri384, ltri256, ltri128], axis=1)).astype(BF)


def make_in_maps(x, c_attn_w, c_proj_w, cached_k, cached_v, att_prefix, cache_v, start_index):
    cos_g, sin_g = _rotary_tables(np.asarray(start_index).item())
    maskw = _mask_const()
    ones64 = np.ones((1, 64), dtype=BF)
    in_maps = []
    for core in range(NCORES):
        b, hg = core // 2, core % 2
        hs = slice(hg * HPC, (hg + 1) * HPC)
        r0, r1 = hg * 512, (hg + 1) * 512
        wq = c_attn_w[r0:r1]
        wk = c_attn_w[C + r0:C + r1]
        wv = c_attn_w[2 * C + r0:2 * C + r1]
        # m-tile-major qk weights: [p, mslot, c, 128] for _MORDER slots
        wqk = np.empty((128, 8, 8, 128), dtype=BF)
        for slot, m in enumerate(_MORDER):
            w_ = (wq if m < 4 else wk)[(m % 4) * 128:(m % 4) * 128 + 128]
            wqk[:, slot] = w_.T.reshape(8, 128, 128).transpose(1, 0, 2)
        w_qka = np.ascontiguousarray(wqk[:, 0:2])
        w_qkb = np.ascontiguousarray(wqk[:, 2:8])
        w_v = np.ascontiguousarray(
            wv.T.reshape(8, 128, 512).transpose(1, 0, 2)).astype(BF)
        xT = np.ascontiguousarray(
            x[b].T.reshape(8, 128, 512).transpose(1, 0, 2)).astype(BF)

        kk = cached_k[b, hs]  # (8, 1536, 64)
        kT = np.empty((128, 4, 1536), dtype=BF)
        for pr in range(4):
            kT[0:64, pr] = kk[2 * pr].T
            kT[64:128, pr] = kk[2 * pr + 1].T

        # v blocks 0..11 with ones col; blocks 12..15 zero except ones col
        vp = cached_v[b, hs].reshape(8, 12, 128, 64).transpose(2, 0, 1, 3)
        vfull = np.zeros((128, 8, 16, 65), np.float32)
        vfull[:, :, 0:12, 0:64] = vp
        vfull[:, :, :, 64] = 1.0
        v_pack = np.ascontiguousarray(vfull.reshape(128, 8, 1040)).astype(BF)

        p_ = att_prefix[b, hs].transpose(0, 2, 1)  # (8, 256, 512)
        prefT = np.ascontiguousarray(
            np.concatenate([p_[:, :128], p_[:, 128:]], axis=2)
            .transpose(1, 0, 2)).astype(BF)  # (128, 8, 1024)

        cv = cache_v[b, hs].reshape(8, 2, 128, 64).transpose(2, 0, 1, 3)
        cvn = np.concatenate(
            [cv, np.ones((128, 8, 2, 1), np.float32)], axis=3).astype(BF)

        w_projT = np.ascontiguousarray(
            c_proj_w[:, r0:r1].T.reshape(4, 128, 1024).transpose(1, 0, 2)
        ).astype(BF)

        in_maps.append({
            "xT": xT,
            "w_qka": w_qka,
            "w_qkb": w_qkb,
            "w_v": w_v,
            "kT_cache": kT,
            "v_pack": v_pack,
            "prefT": prefT,
            "cvn": cvn,
            "w_projT": w_projT,
            "cos_g": cos_g,
            "sin_g": sin_g,
            "maskw": maskw,
            "ones64": ones64,
        })
    return in_maps


_NC_CACHE = {}


def kernel(x, c_attn_w, c_proj_w, cached_k, cached_v, att_prefix, cache_v, start_index):
    x = np.asarray(x, dtype=np.float32)
    c_attn_w = np.asarray(c_attn_w, dtype=np.float32)
    c_proj_w = np.asarray(c_proj_w, dtype=np.float32)
    cached_k = np.asarray(cached_k, dtype=np.float32)
    cached_v = np.asarray(cached_v, dtype=np.float32)
    att_prefix = np.asarray(att_prefix, dtype=np.float32)
    cache_v = np.asarray(cache_v, dtype=np.float32)

    if "nc" not in _NC_CACHE:
        _NC_CACHE["nc"] = build_nc()
    nc = _NC_CACHE["nc"]

    in_maps = make_in_maps(x, c_attn_w, c_proj_w, cached_k, cached_v,
                           att_prefix, cache_v, start_index)
    from concourse.bass_utils import run_bass_kernel_spmd
    res = run_bass_kernel_spmd(nc, in_maps, list(range(NCORES)))
    outs = res.results
    y = np.empty((B, T, C), dtype=np.float32)
    for b in range(B):
        y[b] = (outs[2 * b]["out"].astype(np.float32)
                + outs[2 * b + 1]["out"].astype(np.float32))
    return y
